# revision 1
# baseline (speedup 1.0000x reference)
"""Trainium2 Bass kernel for BipartiteGraphConvolution (right_to_left=False).

    total = max(sum(edge_weight), 1)
    vals  = edge_weight / total
    msg   = left_features[col] * vals[:, None]
    conv  = segment_sum(msg, row, n)
    h     = right_features + temp[1] * (c - conv)
    out   = relu(h @ W1.T + b1) @ W2.T + b2

Strategy (8 NeuronCores, full inputs in / full output out):
  - Shard destination (right) nodes across 8 cores; route edges by row index.
  - Per core, 128-dest blocks. Edges of a block are weighted-one-hot matmul'd
    on the TensorEngine into a PSUM accumulator [64 feats x 128 dests]
    (conv^T), 128 edges per matmul (edges on the contraction axis).
  - Edge source rows are fetched with InstDMAGatherAnt (vectorized Q7 SWDGE
    descriptor generation) on 4 SWDGE queues = all 4 Q7 core pairs in
    parallel. int16 gather indices address a [25000, 128]-bf16 strided view
    of the row-padded table (stride 1024B), one view per col%4 class.
  - Weights (w * temp1/total) ride in the one-hot (built by the VectorEngine
    from per-partition scalars: (iota == row_rel) * w).
  - h^T = right'^T - conv^T on VectorE (right' = right + temp1*c, host-side),
    then the 64x64 MLP in fp32 on TensorE/ScalarE, output written back
    transposed; host untransposes.
"""

import numpy as np
import ml_dtypes

import concourse.bacc as bacc
import concourse.bass as bass
import concourse.mybir as mybir
from concourse.library_config import mlp as _mlp_lib
from concourse.bass_utils import run_bass_kernel_spmd

EMB = 64
N_CORES = 8
_TRACE = False     # set by an external harness to capture an NTFF profile
LAST_RESULT = None
NBUF = 4      # gathered-tile ring (blocks in flight)
NOH = 8       # one-hot ring slots
RROT = 8      # rotating gather sems per queue

_F32 = mybir.dt.float32
_BF16 = mybir.dt.bfloat16
_I16 = mybir.dt.int16


def _preprocess(left_features, edge_index, edge_weight, right_features, c, temp):
    n = right_features.shape[0]
    m = left_features.shape[0]
    D = -(-n // N_CORES)                   # dests per core
    NBLK = -(-D // 128)                    # 128-dest blocks per core
    DP = NBLK * 128                        # padded dests per core

    total = max(float(np.sum(edge_weight, dtype=np.float32)), 1.0)
    scale = np.float32(temp[1]) / np.float32(total)

    rows = np.ascontiguousarray(edge_index[:, 0]).astype(np.int64)
    cols = np.ascontiguousarray(edge_index[:, 1]).astype(np.int64)
    ws = (edge_weight.astype(np.float32) * scale).astype(np.float32)

    core = rows // D
    r_loc = rows - core * D
    blk = r_loc >> 7
    grp = cols & 3

    key = ((core * NBLK + blk) * 4 + grp).astype(np.int64)
    order = np.argsort(key, kind="stable")
    key_s = key[order]
    cnt = np.bincount(key_s, minlength=N_CORES * NBLK * 4)

    S = max(1, -(-int(cnt.max()) // 128))  # 128-slot chunks per (blk, grp)
    SLOT = S * 128
    C = 4 * S                              # chunks per block

    # position of each edge inside its (core, blk, grp) cell
    starts = np.concatenate(([0], np.cumsum(cnt)[:-1]))
    within = np.arange(len(order)) - starts[key_s]
    slot = key_s * SLOT + within           # destination slot, cell-major

    n_cells = N_CORES * NBLK * 4
    idx_pad = np.full(n_cells * SLOT, -1, np.int16)
    w_pad = np.zeros(n_cells * SLOT, np.float32)
    rr_pad = np.zeros(n_cells * SLOT, np.float32)

    idx_pad[slot] = (cols[order] >> 2).astype(np.int16)
    w_pad[slot] = ws[order]
    rr_pad[slot] = (r_loc[order] - blk[order] * 128).astype(np.float32)

    # gather idx tensor per core: [128, NBLK*4*SLOT//16] int16, value i of a
    # gather at [i%16, i//16], replicated 8x down the partitions
    idx16 = idx_pad.reshape(N_CORES, NBLK * 4, SLOT // 16, 16)
    idx16 = np.ascontiguousarray(idx16.transpose(0, 3, 1, 2)).reshape(
        N_CORES, 16, NBLK * 4 * (SLOT // 16))
    idx16 = np.tile(idx16, (1, 8, 1))      # [NC, 128, cols]

    # host-built weighted one-hots, streamed to the device:
    # oh[core, slot(=chunk*128+p), dest_rel] = w_e
    n_chunks = NBLK * C
    oh = np.zeros(N_CORES * n_chunks * 128 * 128, ml_dtypes.bfloat16)
    oh[slot * 128 + (r_loc[order] - blk[order] * 128)] = w_pad[slot]
    # -> [NC, 128(p), n_chunks*128(d)] partition-major for DMA
    oh = np.ascontiguousarray(
        oh.reshape(N_CORES, n_chunks, 128, 128).transpose(0, 2, 1, 3)
    ).reshape(N_CORES, 128, n_chunks * 128)

    # row-padded bf16 table [m4*4, 128] so each row is 256B; view g strides 4
    m4 = -(-m // 4)
    tabp = np.zeros((m4 * 4, 128), ml_dtypes.bfloat16)
    tabp[:m, :EMB] = left_features.astype(ml_dtypes.bfloat16)

    # right' = right + temp1*c, transposed per core [64, DP] f32
    rp = right_features.astype(np.float32) + np.float32(temp[1]) * c.astype(np.float32)
    rp_pad = np.zeros((N_CORES * DP, EMB), np.float32)
    for cc in range(N_CORES):
        lo, hi = cc * D, min((cc + 1) * D, n)
        rp_pad[cc * DP: cc * DP + (hi - lo)] = rp[lo:hi]
    rpT = np.ascontiguousarray(
        rp_pad.reshape(N_CORES, DP, EMB).transpose(0, 2, 1))  # [NC, 64, DP]

    gcnt = np.ascontiguousarray(
        cnt.reshape(N_CORES, 1, NBLK * 4).astype(np.int32))  # [NC, 1, NGATH]

    meta = dict(n=n, m=m, m4=m4, D=D, NBLK=NBLK, DP=DP, S=S, SLOT=SLOT, C=C,
                n_chunks=n_chunks)
    return meta, dict(tab=tabp, idx16=idx16, oh=oh, rpT=rpT, gcnt=gcnt)


def _build(meta, W1, b1, W2, b2):
    import time as _time
    _t0 = _time.time()
    NBLK, S, SLOT, C = meta["NBLK"], meta["S"], meta["SLOT"], meta["C"]
    DP, m4 = meta["DP"], meta["m4"]
    n_chunks = meta["n_chunks"]
    IDXC = NBLK * 4 * (SLOT // 16)

    nc = bacc.Bacc("TRN2", num_swdge_queues=4)

    tab = nc.declare_dram_parameter("tab", [m4 * 4, 128], _BF16, isOutput=False)
    idx16 = nc.declare_dram_parameter("idx16", [128, IDXC], _I16, isOutput=False)
    oh_d = nc.declare_dram_parameter("oh", [128, n_chunks * 128], _BF16,
                                     isOutput=False)
    rpT = nc.declare_dram_parameter("rpT", [EMB, DP], _F32, isOutput=False)
    w1t_d = nc.declare_dram_parameter("w1t", [EMB, EMB], _F32, isOutput=False)
    w2t_d = nc.declare_dram_parameter("w2t", [EMB, EMB], _F32, isOutput=False)
    b1_d = nc.declare_dram_parameter("b1", [EMB, 1], _F32, isOutput=False)
    b2_d = nc.declare_dram_parameter("b2", [EMB, 1], _F32, isOutput=False)
    gcnt_d = nc.declare_dram_parameter("gcnt", [1, NBLK * 4], mybir.dt.int32,
                                       isOutput=False)
    outT = nc.declare_dram_parameter("outT", [EMB, DP], _F32, isOutput=True)

    tab_v = tab[:].rearrange("(n r) e -> r n e", r=4)  # [4, m4, 128]

    import contextlib
    ctx = contextlib.ExitStack()
    with ctx:
        idx_sb = ctx.enter_context(nc.sbuf_tensor([128, IDXC], _I16))
        w1t_sb = ctx.enter_context(nc.sbuf_tensor([EMB, EMB], _F32))
        w2t_sb = ctx.enter_context(nc.sbuf_tensor([EMB, EMB], _F32))
        b1_sb = ctx.enter_context(nc.sbuf_tensor([EMB, 1], _F32))
        b2_sb = ctx.enter_context(nc.sbuf_tensor([EMB, 1], _F32))
        gcnt_sb = ctx.enter_context(nc.sbuf_tensor([1, NBLK * 4], mybir.dt.int32))
        ring = [ctx.enter_context(nc.sbuf_tensor(f"ring{i}", [128, C, 128], _BF16))
                for i in range(NBUF)]
        ohblk = [ctx.enter_context(nc.sbuf_tensor(f"ohblk{i}", [128, C, 128], _BF16))
                 for i in range(2)]
        rpT_sb = [ctx.enter_context(nc.sbuf_tensor(f"rpT_sb{i}", [EMB, 128], _F32))
                  for i in range(2)]
        hT_sb = [ctx.enter_context(nc.sbuf_tensor(f"hT_sb{i}", [EMB, 128], _F32))
                 for i in range(2)]
        hr_sb = [ctx.enter_context(nc.sbuf_tensor(f"hr_sb{i}", [EMB, 128], _F32))
                 for i in range(2)]
        oT_sb = [ctx.enter_context(nc.sbuf_tensor(f"oT_sb{i}", [EMB, 128], _F32))
                 for i in range(2)]
        acc_ps = [ctx.enter_context(nc.psum_tensor(f"acc_ps{i}", [128, 512], _F32))
                  for i in range(2)]
        mm1_ps = [ctx.enter_context(nc.psum_tensor(f"mm1_ps{i}", [128, 512], _F32))
                  for i in range(2)]
        mm2_ps = [ctx.enter_context(nc.psum_tensor(f"mm2_ps{i}", [128, 512], _F32))
                  for i in range(2)]

        ld = ctx.enter_context(nc.semaphore())
        rp_sems = [ctx.enter_context(nc.semaphore(f"rp{i}")) for i in range(2)]
        oh_sems = [ctx.enter_context(nc.semaphore(f"oh{i}")) for i in range(2)]
        t_s = ctx.enter_context(nc.semaphore())
        hv_s = ctx.enter_context(nc.semaphore())
        pm1 = ctx.enter_context(nc.semaphore())
        a1 = ctx.enter_context(nc.semaphore())
        pm2 = ctx.enter_context(nc.semaphore())
        a2 = ctx.enter_context(nc.semaphore())
        od_sems = [ctx.enter_context(nc.semaphore(f"od{i}")) for i in range(2)]
        ms_s = ctx.enter_context(nc.semaphore())
        gq = [[ctx.enter_context(nc.semaphore(f"gq{q}_{r}")) for r in range(RROT)]
              for q in range(4)]

        blk = ctx.enter_context(nc.Block())

        @blk.sync
        def _(sy):
            sy.dma_start(out=idx_sb[:], in_=idx16[:]).then_inc(ld, 16)
            sy.dma_start(out=w1t_sb[:], in_=w1t_d[:]).then_inc(ld, 16)
            sy.dma_start(out=w2t_sb[:], in_=w2t_d[:]).then_inc(ld, 16)
            sy.dma_start(out=b1_sb[:], in_=b1_d[:]).then_inc(ld, 16)
            sy.dma_start(out=b2_sb[:], in_=b2_d[:]).then_inc(ld, 16)
            sy.dma_start(out=gcnt_sb[:], in_=gcnt_d[:]).then_inc(ld, 16)
            for b in range(NBLK + 2):
                if b < NBLK:
                    if b >= 2:
                        sy.wait_ge(hv_s, b - 1)
                    sy.dma_start(out=rpT_sb[b % 2][:],
                                 in_=rpT[:, b * 128:(b + 1) * 128]
                                 ).then_inc(rp_sems[b % 2], 16)
                    sy.dma_start(out=ohblk[b % 2][:].rearrange("p c e -> p (c e)"),
                                 in_=oh_d[:, b * C * 128:(b + 1) * C * 128]
                                 ).then_inc(oh_sems[b % 2], 16)
                if b >= 2:
                    sy.wait_ge(a2, b - 1)
                    sy.dma_start(out=outT[:, (b - 2) * 128:(b - 1) * 128],
                                 in_=oT_sb[b % 2][:]).then_inc(od_sems[b % 2], 16)
            sy.wait_ge(od_sems[0], 16 * ((NBLK + 1) // 2))
            sy.wait_ge(od_sems[1], 16 * (NBLK // 2))

        @blk.gpsimd
        def _(g):
            cnt_r = g.alloc_register("gcnt_r")
            g.load_library(_mlp_lib)
            g.wait_ge(ld, 96)  # preamble loaded
            g.wait_ge(ms_s, NBUF)  # rings memset (NaN guard for skipped slots)
            for b in range(NBLK):
                if b >= NBUF:
                    g.wait_ge(t_s, C * (b - NBUF + 1))
                for q in range(4):
                    off = (b * 4 + q) * (SLOT // 16)
                    g.reg_load(cnt_r, gcnt_sb[0:1, b * 4 + q:b * 4 + q + 1])
                    g.dma_gather(
                        ring[b % NBUF][:, q * S:(q + 1) * S, :],
                        tab_v[q],
                        idx_sb[:, off:off + SLOT // 16],
                        SLOT, cnt_r, 128,
                        elem_step=512,
                        single_packet=False,
                        queue_num=q,
                    ).then_inc(gq[q][b % RROT], 16)

        @blk.vector
        def _(v):
            for s in range(NBUF):
                v.memset(ring[s][:].rearrange("p c e -> p (c e)"), 0).then_inc(ms_s, 1)
            v.wait_ge(ld, 96)  # preamble loaded
            for b in range(NBLK):
                # h^T(b) = rp^T(b) - conv^T(b)
                v.wait_ge(t_s, C * (b + 1))
                v.wait_ge(rp_sems[b % 2], 16 * (b // 2 + 1))
                if b >= 2:
                    v.wait_ge(pm1, b - 1)  # hT[b%2] consumed by mm1(b-2)
                v.tensor_tensor(
                    out=hT_sb[b % 2][:],
                    in0=rpT_sb[b % 2][:],
                    in1=acc_ps[b % 2][0:EMB, 0:128],
                    op=mybir.AluOpType.subtract,
                ).then_inc(hv_s, 1)

        @blk.tensor
        def _(t):
            t.wait_ge(ld, 96)

            def chunks(b):
                for q in range(4):
                    t.wait_ge(gq[q][b % RROT], 16 * (b // RROT + 1))
                t.wait_ge(oh_sems[b % 2], 16 * (b // 2 + 1))
                if b >= 2:
                    t.wait_ge(hv_s, b - 1)  # acc_ps[b%2] free
                for k in range(C):
                    t.matmul(
                        out=acc_ps[b % 2][0:EMB, 0:128],
                        lhsT=ring[b % NBUF][:, k, 0:EMB],
                        rhs=ohblk[b % 2][:, k, :],
                        start=(k == 0),
                        stop=(k == C - 1),
                    ).then_inc(t_s, 1)

            def mm1(b):
                t.wait_ge(hv_s, b + 1)
                if b >= 2:
                    t.wait_ge(a1, b - 1)  # mm1_ps[b%2] free
                t.matmul(out=mm1_ps[b % 2][0:EMB, 0:128], lhsT=w1t_sb[:],
                         rhs=hT_sb[b % 2][:], start=True, stop=True,
                         ).then_inc(pm1, 1)

            def mm2(b):
                t.wait_ge(a1, b + 1)
                if b >= 2:
                    t.wait_ge(a2, b - 1)  # mm2_ps[b%2] free
                t.matmul(out=mm2_ps[b % 2][0:EMB, 0:128], lhsT=w2t_sb[:],
                         rhs=hr_sb[b % 2][:], start=True, stop=True,
                         ).then_inc(pm2, 1)

            for b in range(NBLK + 2):
                if b < NBLK:
                    chunks(b)
                if 1 <= b < NBLK + 1:
                    mm1(b - 1)
                if b >= 2:
                    mm2(b - 2)

        @blk.scalar
        def _(sc):
            sc.wait_ge(ld, 96)
            for b in range(NBLK):
                # relu(mm1 + b1)
                sc.wait_ge(pm1, b + 1)
                if b >= 2:
                    sc.wait_ge(pm2, b - 1)  # hr_sb[b%2] consumed by mm2(b-2)
                sc.activation(out=hr_sb[b % 2][:], in_=mm1_ps[b % 2][0:EMB, 0:128],
                              func=mybir.ActivationFunctionType.Relu,
                              bias=b1_sb[:]).then_inc(a1, 1)
                # out = mm2 + b2
                sc.wait_ge(pm2, b + 1)
                if b >= 2:
                    sc.wait_ge(od_sems[b % 2], 16 * (b // 2))  # oT_sb[b%2] stored
                sc.activation(out=oT_sb[b % 2][:], in_=mm2_ps[b % 2][0:EMB, 0:128],
                              func=mybir.ActivationFunctionType.Identity,
                              bias=b2_sb[:]).then_inc(a2, 1)

    print(f"[kernel] trace built in {_time.time()-_t0:.1f}s; compiling...", flush=True)
    _t1 = _time.time()
    nc.compile()
    print(f"[kernel] bacc compile: {_time.time()-_t1:.1f}s", flush=True)
    return nc


def kernel(left_features, right_features_k, edge_index, edge_weight,
           right_features, c, b, temp, W1, b1, W2, b2):
    import time as _time
    n = right_features.shape[0]
    _t0 = _time.time()
    meta, arrs = _preprocess(left_features, edge_index, edge_weight,
                             right_features, c, temp)
    print(f"[kernel] preprocess: {_time.time()-_t0:.1f}s meta={meta}", flush=True)
    nc = _build(meta, W1, b1, W2, b2)

    w1t = np.ascontiguousarray(W1.astype(np.float32).T)
    w2t = np.ascontiguousarray(W2.astype(np.float32).T)
    b1c = np.ascontiguousarray(b1.astype(np.float32).reshape(EMB, 1))
    b2c = np.ascontiguousarray(b2.astype(np.float32).reshape(EMB, 1))

    in_maps = []
    for cc in range(N_CORES):
        in_maps.append({
            "tab": arrs["tab"],
            "idx16": np.ascontiguousarray(arrs["idx16"][cc]),
            "oh": arrs["oh"][cc],
            "rpT": np.ascontiguousarray(arrs["rpT"][cc]),
            "gcnt": np.ascontiguousarray(arrs["gcnt"][cc]),
            "w1t": w1t,
            "w2t": w2t,
            "b1": b1c,
            "b2": b2c,
        })

    global LAST_RESULT
    _t2 = _time.time()
    res = run_bass_kernel_spmd(nc, in_maps, list(range(N_CORES)), trace=_TRACE)
    print(f"[kernel] run (incl neff compile+exec): {_time.time()-_t2:.1f}s", flush=True)
    LAST_RESULT = res

    D, DP = meta["D"], meta["DP"]
    out = np.empty((n, EMB), np.float32)
    for cc in range(N_CORES):
        lo, hi = cc * D, min((cc + 1) * D, n)
        oT = res.results[cc]["outT"]          # [64, DP]
        out[lo:hi] = oT.T[: hi - lo]
    return out

